# revision 1
# baseline (speedup 1.0000x reference)
"""Cached scaled-dot-product-attention decode kernel for Trainium2 (Bass/Tile).

Full inputs -> shard batch across 8 NeuronCores (B=8, one batch per core)
-> per-core Bass kernel computes, for each of its 32 heads:
    K = cache_k[h] with row cache_pos replaced by key[h]
    V = cache_v[h] with row cache_pos replaced by value[h]
    out[h] = softmax(q K^T / sqrt(D)) V        (over the first cache_pos+1 rows)
-> gather per-core outputs into the full [B, H, 1, D] array.

Layout trick: cache_k[h] ([S, D] row-major in HBM) is loaded as SBUF
[128, S] via "(p r) d -> p (r d)" so every partition reads one fully
contiguous 16KB chunk (max DMA efficiency).  Sequence position
s = p*R + r lands at (partition p, column-block r).  This is a fixed
permutation of the sequence axis, which softmax(..)V is invariant to, as
long as K and V use the same permutation (they do).

Scores are computed on the DVE (one big elementwise multiply against a
partition-broadcast q, then a 3D tensor_reduce over d) so K never needs
a transpose.  attn@V contracts over the partition axis on the PE
(lhsT = prob column, rhs = natural V tile); softmax normalization is a
single reciprocal + tensor_scalar at the end (exp is unshifted — scores
are ~N(0,1) so fp32 exp cannot overflow).
"""

import math
from contextlib import ExitStack

import numpy as np

import concourse.bacc as bacc
import concourse.mybir as mybir
import concourse.tile as tile
from concourse.bass_utils import run_bass_kernel_spmd

F32 = mybir.dt.float32
BF16 = mybir.dt.bfloat16

N_CORES = 8

_program_cache: dict = {}
_last_results = None


def _build(H: int, S: int, D: int, cache_pos: int):
    """Build + compile the per-core Bass program (identical on all cores)."""
    P = 128
    R = S // P  # column blocks / rows-per-partition (32 for S=4096)
    assert S % P == 0 and D == 128
    end_pos = cache_pos + 1
    scale = 1.0 / math.sqrt(D)

    nc = bacc.Bacc(
        "TRN2",
        target_bir_lowering=False,
        debug=False,
        enable_asserts=False,
        num_devices=N_CORES,
    )
    q_d = nc.dram_tensor("query", [H, 1, D], F32, kind="ExternalInput").ap()
    k_d = nc.dram_tensor("key", [H, 1, D], F32, kind="ExternalInput").ap()
    v_d = nc.dram_tensor("value", [H, 1, D], F32, kind="ExternalInput").ap()
    ck_d = nc.dram_tensor("cache_k", [H, S, D], F32, kind="ExternalInput").ap()
    cv_d = nc.dram_tensor("cache_v", [H, S, D], F32, kind="ExternalInput").ap()
    out_d = nc.dram_tensor("out", [1, H * D], F32, kind="ExternalOutput").ap()

    pp = cache_pos // R  # partition holding the patched row
    rr = cache_pos % R  # column block holding the patched row

    with tile.TileContext(nc) as tc, ExitStack() as ctx:
        const_pool = ctx.enter_context(tc.tile_pool(name="const", bufs=1))
        kv_pool = ctx.enter_context(tc.tile_pool(name="kv", bufs=3))
        sm_pool = ctx.enter_context(tc.tile_pool(name="sm", bufs=2))
        ps_build = ctx.enter_context(tc.tile_pool(name="psb", bufs=2, space="PSUM"))
        ps_av = ctx.enter_context(tc.tile_pool(name="psav", bufs=2, space="PSUM"))
        ps_z = ctx.enter_context(tc.tile_pool(name="psz", bufs=2, space="PSUM"))

        ones_t = const_pool.tile([P, P], F32, name="ones_t")
        nc.vector.memset(ones_t[:], 1.0)
        ones_row = ones_t[0:1, :]
        ones_col = ones_t[:, 0:1]

        out_stage = const_pool.tile([1, H * D], F32, name="out_stage")
        # out_stage doubles as the q staging row during the prologue (it is
        # only written by the per-head epilogues, which depend on q_bc).
        q_flat = out_stage
        nc.sync.dma_start(q_flat[:], q_d.rearrange("h q d -> q (h d)"))
        q_bc = const_pool.tile([P, H * D], F32, name="q_bc")
        NB = 512
        for j in range((H * D + NB - 1) // NB):
            nb = min(NB, H * D - j * NB)
            qb_ps = ps_build.tile([P, NB], F32, name="qb_ps")
            nc.tensor.matmul(
                qb_ps[:, :nb],
                ones_row[:],
                q_flat[0:1, j * NB : j * NB + nb],
                start=True,
                stop=True,
            )
            # fold the 1/sqrt(D) softmax scale into the broadcast copy
            nc.scalar.mul(q_bc[:, j * NB : j * NB + nb], qb_ps[:, :nb], scale)

        mask = None
        if end_pos < S:
            # Additive score mask: 0 where s = p*R + r < end_pos, -1e30 after.
            s_iota = const_pool.tile([P, R], F32, name="s_iota")
            nc.gpsimd.iota(
                s_iota[:],
                [[1, R]],
                channel_multiplier=R,
                allow_small_or_imprecise_dtypes=True,
            )
            mask = const_pool.tile([P, R], F32, name="mask")
            nc.vector.tensor_scalar(
                mask[:],
                s_iota[:],
                float(end_pos),
                -1e30,
                op0=mybir.AluOpType.is_ge,
                op1=mybir.AluOpType.mult,
            )

        for h in range(H):
            # The last head's chain (mult -> reduce -> exp -> attn@V) is the
            # kernel's drain tail: split its stages in halves so each stage
            # overlaps the second half of its K load. Other heads stay whole
            # (splitting every load costs DMA descriptor efficiency).
            nsplit = 2 if h == H - 1 else 1
            RC, SC = R // nsplit, S // nsplit

            # All-fp32 numerics. K loads ride the HWDGE (sync) ring, V loads
            # the SWDGE (gpsimd) ring — splitting the two 2MiB streams across
            # both descriptor-generation paths keeps the SDMA engines fed.
            k_t = kv_pool.tile([P, S], F32, name="k_t", tag="k")
            ck_h = ck_d[h].rearrange("(p r) d -> p (r d)", p=P)
            for c in range(nsplit):
                nc.sync.dma_start(
                    k_t[:, c * SC : (c + 1) * SC], ck_h[:, c * SC : (c + 1) * SC]
                )
            v_t = kv_pool.tile([P, S], F32, name="v_t", tag="v")
            nc.gpsimd.dma_start(v_t[:], cv_d[h].rearrange("(p r) d -> p (r d)", p=P))
            # scatter the decode-step key/value into the loaded cache tiles
            nc.sync.dma_start(k_t[pp : pp + 1, rr * D : (rr + 1) * D], k_d[h])
            nc.gpsimd.dma_start(v_t[pp : pp + 1, rr * D : (rr + 1) * D], v_d[h])

            # scores[p, r] = sum_d K[p, r, d] * q_scaled[d]   for s = p*R + r
            scores = sm_pool.tile([P, R], F32, name="scores", tag="scores")
            prod = sm_pool.tile([P, S], F32, name="prod", tag="prod", bufs=1)
            p_t = sm_pool.tile([P, R], F32, name="p_t", tag="p")
            z_cols = []
            for c in range(nsplit):
                qh = (
                    q_bc[:, h * D : (h + 1) * D]
                    .rearrange("p (o d) -> p o d", o=1)
                    .broadcast_to([P, RC, D])
                )
                k3 = k_t[:, c * SC : (c + 1) * SC].rearrange("p (r d) -> p r d", r=RC)
                prod3 = prod[:, c * SC : (c + 1) * SC].rearrange(
                    "p (r d) -> p r d", r=RC
                )
                sc_c = scores[:, c * RC : (c + 1) * RC]
                nc.vector.tensor_tensor(prod3, k3, qh, op=mybir.AluOpType.mult)
                nc.vector.tensor_reduce(
                    sc_c, prod3, axis=mybir.AxisListType.X, op=mybir.AluOpType.add
                )
                if mask is not None:
                    nc.vector.tensor_tensor(
                        sc_c,
                        sc_c,
                        mask[:, c * RC : (c + 1) * RC],
                        op=mybir.AluOpType.add,
                    )
                # p = exp(scores); z_col[p] = partial softmax denominator
                z_col = sm_pool.tile([P, 1], F32, name="z_col", tag=f"z{c}")
                nc.scalar.activation(
                    p_t[:, c * RC : (c + 1) * RC],
                    sc_c,
                    mybir.ActivationFunctionType.Exp,
                    accum_out=z_col[:],
                )
                z_cols.append(z_col)

            # out_unnorm[1, D] = sum_r p[:, r]^T @ V_tile_r  (emitted before
            # the Z matmuls so PE starts attn@V as soon as exp halves land)
            av_ps = ps_av.tile([1, D], F32, name="av_ps")
            for r in range(R):
                nc.tensor.matmul(
                    av_ps[:],
                    p_t[:, r : r + 1],
                    v_t[:, r * D : (r + 1) * D],
                    start=(r == 0),
                    stop=(r == R - 1),
                )
            # Z = sum over partitions of the z_col partials (contract on PE)
            z_ps = ps_z.tile([1, 1], F32, name="z_ps")
            for c, z_col in enumerate(z_cols):
                nc.tensor.matmul(
                    z_ps[:],
                    z_col[:],
                    ones_col[:],
                    start=(c == 0),
                    stop=(c == len(z_cols) - 1),
                )
            rz = sm_pool.tile([1, 1], F32, name="rz", tag="rz")
            nc.vector.reciprocal(rz[:], z_ps[:])
            nc.vector.tensor_scalar_mul(
                out_stage[0:1, h * D : (h + 1) * D], av_ps[:], rz[:]
            )

        nc.sync.dma_start(out_d[:], out_stage[:])

    nc.compile()
    return nc


def _get_program(H, S, D, cache_pos):
    key = (H, S, D, cache_pos)
    if key not in _program_cache:
        _program_cache[key] = _build(H, S, D, cache_pos)
    return _program_cache[key]


def kernel(query, key, value, cache_k, cache_v, cache_pos):
    cache_pos = int(cache_pos)
    B, H, Q, D = query.shape
    S = cache_k.shape[2]
    assert Q == 1 and B == N_CORES

    nc = _get_program(H, S, D, cache_pos)

    f32 = np.float32
    in_maps = [
        {
            "query": np.ascontiguousarray(query[b], dtype=f32),
            "key": np.ascontiguousarray(key[b], dtype=f32),
            "value": np.ascontiguousarray(value[b], dtype=f32),
            "cache_k": np.ascontiguousarray(cache_k[b], dtype=f32),
            "cache_v": np.ascontiguousarray(cache_v[b], dtype=f32),
        }
        for b in range(B)
    ]
    res = run_bass_kernel_spmd(nc, in_maps, core_ids=list(range(N_CORES)))
    global _last_results
    _last_results = res
    out = np.stack(
        [res.results[b]["out"].reshape(H, 1, D).astype(np.float32) for b in range(B)]
    )
    return out



# revision 3
# speedup vs baseline: 1.1658x; 1.1658x over previous
"""Cached scaled-dot-product-attention decode kernel for Trainium2 (Bass/Tile).

Full inputs -> shard batch across 8 NeuronCores (B=8, one batch per core)
-> per-core Bass kernel computes, for each of its 32 heads:
    out[h] = softmax(q K^T / sqrt(D)) V     over the cache's valid prefix
-> gather per-core outputs into the full [B, H, 1, D] array.

The decode-step key/value row is patched into the per-core cache copies on
the HOST (kernel() already materialises per-core contiguous copies), so the
device program streams pure cache and never does scatter DMAs.

Layout trick: cache_k[h] ([S, D] row-major in HBM) is loaded as SBUF
[128, S] via "(p r) d -> p (r d)" so every partition reads one fully
contiguous 16KB chunk (max DMA efficiency).  Sequence position s = p*R + r
lands at (partition p, column-block r) — a fixed permutation of the
sequence axis, which softmax(..)V is invariant to as long as K and V use
the same permutation (they do).

All on-chip compute is fp16 (cast during the SWDGE DMA load): DVE
tensor_tensor runs in 2x_1p mode and PE matmuls stream at 1 cycle/row
(4x faster than fp32).  Score sums and softmax accumulators stay fp32.
"""

import math
from contextlib import ExitStack

import numpy as np

import concourse.bacc as bacc
import concourse.mybir as mybir
import concourse.tile as tile
from concourse.bass_utils import run_bass_kernel_spmd

F32 = mybir.dt.float32
FP16 = mybir.dt.float16

N_CORES = 8

_program_cache: dict = {}
_last_results = None


def _build(H: int, S: int, D: int, cache_pos: int):
    """Build + compile the per-core Bass program (identical on all cores)."""
    P = 128
    R = S // P  # column blocks / rows-per-partition (32 for S=4096)
    assert S % P == 0 and D == 128
    end_pos = cache_pos + 1
    scale = 1.0 / math.sqrt(D)

    nc = bacc.Bacc(
        "TRN2",
        target_bir_lowering=False,
        debug=False,
        enable_asserts=False,
        num_devices=N_CORES,
    )
    q_d = nc.dram_tensor("query", [H, 1, D], F32, kind="ExternalInput").ap()
    ck_d = nc.dram_tensor("cache_k", [H, S, D], F32, kind="ExternalInput").ap()
    cv_d = nc.dram_tensor("cache_v", [H, S, D], F32, kind="ExternalInput").ap()
    out_d = nc.dram_tensor("out", [1, H * D], F32, kind="ExternalOutput").ap()

    with tile.TileContext(nc) as tc, ExitStack() as ctx:
        const_pool = ctx.enter_context(tc.tile_pool(name="const", bufs=1))
        kv_pool = ctx.enter_context(tc.tile_pool(name="kv", bufs=5))
        sm_pool = ctx.enter_context(tc.tile_pool(name="sm", bufs=2))
        ps_build = ctx.enter_context(tc.tile_pool(name="psb", bufs=2, space="PSUM"))
        ps_av = ctx.enter_context(tc.tile_pool(name="psav", bufs=2, space="PSUM"))
        ps_z = ctx.enter_context(tc.tile_pool(name="psz", bufs=2, space="PSUM"))

        ones_h = const_pool.tile([1, P], FP16, name="ones_h")
        nc.vector.memset(ones_h[:], 1.0)
        ones_col = const_pool.tile([P, 1], F32, name="ones_col")
        nc.vector.memset(ones_col[:], 1.0)

        out_stage = const_pool.tile([1, H * D], F32, name="out_stage")

        # q: load fp16 (SWDGE cast), broadcast to 128 partitions on the PE,
        # folding the 1/sqrt(D) softmax scale into the PSUM->SBUF copy.
        q_flat = const_pool.tile([1, H * D], FP16, name="q_flat")
        nc.gpsimd.dma_start(q_flat[:], q_d.rearrange("h q d -> q (h d)"))
        q_bc = const_pool.tile([P, H * D], FP16, name="q_bc")
        NB = 512
        for j in range((H * D + NB - 1) // NB):
            nb = min(NB, H * D - j * NB)
            qb_ps = ps_build.tile([P, NB], F32, name="qb_ps")
            nc.tensor.matmul(
                qb_ps[:, :nb],
                ones_h[:],
                q_flat[0:1, j * NB : j * NB + nb],
                start=True,
                stop=True,
            )
            nc.scalar.mul(q_bc[:, j * NB : j * NB + nb], qb_ps[:, :nb], scale)

        mask = None
        if end_pos < S:
            # Additive score mask: 0 where s = p*R + r < end_pos, else -30000
            # (safely past fp16 exp underflow; scores are O(1)).
            s_iota = const_pool.tile([P, R], F32, name="s_iota")
            nc.gpsimd.iota(
                s_iota[:],
                [[1, R]],
                channel_multiplier=R,
                allow_small_or_imprecise_dtypes=True,
            )
            mask = const_pool.tile([P, R], F32, name="mask")
            nc.vector.tensor_scalar(
                mask[:],
                s_iota[:],
                float(end_pos),
                -30000.0,
                op0=mybir.AluOpType.is_ge,
                op1=mybir.AluOpType.mult,
            )

        for h in range(H):
            # The last heads' chains (mult -> reduce -> exp -> attn@V) are the
            # kernel's drain tail: split their stages so each stage overlaps
            # the rest of its K/V load. Other heads stay whole (splitting
            # every load costs DMA descriptor efficiency).
            nsplit = 4 if h == H - 1 else (2 if h == H - 2 else 1)
            RC, SC = R // nsplit, S // nsplit

            # fp16 tiles, cast during the SWDGE (gpsimd-ring) DMA load.
            # K/V chunks interleaved so split heads' compute chains start
            # as soon as their first chunks land.
            k_t = kv_pool.tile([P, S], FP16, name="k_t", tag="k")
            v_t = kv_pool.tile([P, S], FP16, name="v_t", tag="v")
            ck_h = ck_d[h].rearrange("(p r) d -> p (r d)", p=P)
            cv_h = cv_d[h].rearrange("(p r) d -> p (r d)", p=P)
            for c in range(nsplit):
                nc.gpsimd.dma_start(
                    k_t[:, c * SC : (c + 1) * SC], ck_h[:, c * SC : (c + 1) * SC]
                )
                nc.gpsimd.dma_start(
                    v_t[:, c * SC : (c + 1) * SC], cv_h[:, c * SC : (c + 1) * SC]
                )

            # scores[p, r] = sum_d K[p, r, d] * q_scaled[d]   for s = p*R + r
            scores = sm_pool.tile([P, R], F32, name="scores", tag="scores")
            prod = sm_pool.tile([P, S], FP16, name="prod", tag="prod", bufs=1)
            p_t = sm_pool.tile([P, R], FP16, name="p_t", tag="p")
            av_ps = ps_av.tile([1, D], F32, name="av_ps")
            z_ps = ps_z.tile([1, 1], F32, name="z_ps")
            for c in range(nsplit):
                qh = (
                    q_bc[:, h * D : (h + 1) * D]
                    .rearrange("p (o d) -> p o d", o=1)
                    .broadcast_to([P, RC, D])
                )
                k3 = k_t[:, c * SC : (c + 1) * SC].rearrange("p (r d) -> p r d", r=RC)
                prod3 = prod[:, c * SC : (c + 1) * SC].rearrange(
                    "p (r d) -> p r d", r=RC
                )
                sc_c = scores[:, c * RC : (c + 1) * RC]
                nc.vector.tensor_tensor(prod3, k3, qh, op=mybir.AluOpType.mult)
                nc.vector.tensor_reduce(
                    sc_c, prod3, axis=mybir.AxisListType.X, op=mybir.AluOpType.add
                )
                if mask is not None:
                    nc.vector.tensor_tensor(
                        sc_c,
                        sc_c,
                        mask[:, c * RC : (c + 1) * RC],
                        op=mybir.AluOpType.add,
                    )
                # p = exp(scores); z_col[p] = partial softmax denominator.
                # Unshifted exp is safe: scores are ~N(0,1).
                z_col = sm_pool.tile([P, 1], F32, name="z_col", tag=f"z{c}")
                nc.scalar.activation(
                    p_t[:, c * RC : (c + 1) * RC],
                    sc_c,
                    mybir.ActivationFunctionType.Exp,
                    accum_out=z_col[:],
                )
                # out_unnorm[1, D] += p[:, r]^T @ V_tile_r  (fp16, 1 cyc/row)
                for r in range(c * RC, (c + 1) * RC):
                    nc.tensor.matmul(
                        av_ps[:],
                        p_t[:, r : r + 1],
                        v_t[:, r * D : (r + 1) * D],
                        start=(r == 0),
                        stop=(r == R - 1),
                    )
                # Z += partition-sum of this chunk's z_col (contract on PE)
                nc.tensor.matmul(
                    z_ps[:],
                    z_col[:],
                    ones_col[:],
                    start=(c == 0),
                    stop=(c == nsplit - 1),
                )
            rz = sm_pool.tile([1, 1], F32, name="rz", tag="rz")
            nc.vector.reciprocal(rz[:], z_ps[:])
            # normalize on the (otherwise idle) scalar engine: ACT reads the
            # PSUM row, scales by 1/Z, writes the fp32 output row.
            nc.scalar.mul(out_stage[0:1, h * D : (h + 1) * D], av_ps[:], rz[0:1, 0:1])

            if h == H // 2 - 1:
                nc.sync.dma_start(
                    out_d[0:1, : (H // 2) * D], out_stage[0:1, : (H // 2) * D]
                )
        nc.sync.dma_start(
            out_d[0:1, (H // 2) * D :], out_stage[0:1, (H // 2) * D :]
        )

    nc.compile()
    return nc


def _get_program(H, S, D, cache_pos):
    key = (H, S, D, cache_pos)
    if key not in _program_cache:
        _program_cache[key] = _build(H, S, D, cache_pos)
    return _program_cache[key]


def kernel(query, key, value, cache_k, cache_v, cache_pos):
    cache_pos = int(cache_pos)
    B, H, Q, D = query.shape
    S = cache_k.shape[2]
    assert Q == 1 and B == N_CORES

    nc = _get_program(H, S, D, cache_pos)

    f32 = np.float32
    in_maps = []
    for b in range(B):
        ck = np.array(cache_k[b], dtype=f32)  # per-core contiguous copy
        cv = np.array(cache_v[b], dtype=f32)
        # the torch module's in-place decode-step write, done host-side
        ck[:, cache_pos : cache_pos + Q, :] = key[b]
        cv[:, cache_pos : cache_pos + Q, :] = value[b]
        in_maps.append(
            {
                "query": np.ascontiguousarray(query[b], dtype=f32),
                "cache_k": ck,
                "cache_v": cv,
            }
        )
    res = run_bass_kernel_spmd(nc, in_maps, core_ids=list(range(N_CORES)))
    global _last_results
    _last_results = res
    out = np.stack(
        [res.results[b]["out"].reshape(H, 1, D).astype(np.float32) for b in range(B)]
    )
    return out
